# revision 17
# baseline (speedup 1.0000x reference)
"""Trainium2 Bass kernel for grouped-correlation multi-view warping (MVS similarity).

Computation (original nn.Module): for each source view s, warp src_fea[s] to the
reference view at D depth hypotheses via per-pixel projection, then accumulate
grouped correlation with the reference feature:
    sim_sum[b,g,d,h,w] = sum_s mean_{c in g} warped[s,b,c,d,h,w] * ref[b,c,h,w]

Key structural property of this module's input distribution: the projection
chain composes INTR_INV twice (src_p = INTR_INV @ src_proj, proj = src_R @
ref_R^T, rot = INTR @ proj @ INTR_INV), so for near-identity extrinsics the
effective rotation has ~1e-5 scale and EVERY projected point lands in the
[0,1) x [0,1) pixel cell (or is masked out-of-bounds to exactly (0,0)).
Hence floor(px) == floor(py) == 0 for every pixel/depth/view: the bilinear taps
are always the four corner pixels (0,0),(0,1),(1,0),(1,1) of the source image,
and only the bilinear WEIGHTS (fx=px, fy=py) vary per output element.  This is
verified on the host for the actual inputs (cheap numpy mirror); if it ever
failed we fall back to a full general recomputation on host.

So the kernel computes, per (s,b): DOT[t,g,h,w] = (1/4) sum_{c in g}
ref[b,c,h,w] * src[s,b,c,tap_t]  (device PE/DVE, hoisted out of the depth
loop), then per depth plane the projection chain -> weights -> blend:
    sim[g,d,h,w] = sum_s sum_t w_t[s,d,h,w] * DOT_s[t,g,h,w]

Sharding: 8 cores = 2 batches x 4 depth-quarters (12 planes each); each core
handles both views; outputs are disjoint -> no collectives.
"""

import sys

sys.path.insert(0, "/opt/trn_rl_repo")

import numpy as np

B, C, H, W, D, S, G = 2, 32, 128, 160, 48, 2, 8
HW = H * W
CPG = C // G
NCORES = 8
DQ = D // 4  # depth planes per core

INTR = np.array(
    [[361.54126, 0.0, 102.9005], [0.0, 360.39624, 77.38375], [0.0, 0.0, 1.0]],
    np.float32,
)
INTR_INV = np.array(
    [[0.00276594, 0.0, -0.2846162], [0.0, 0.00277472, -0.21471854], [0.0, 0.0, 1.0]],
    np.float32,
)

_PROGRAM_CACHE = {}


def _build_program():
    if "nc" in _PROGRAM_CACHE:
        return _PROGRAM_CACHE["nc"]

    import concourse.bacc as bacc
    import concourse.mybir as mybir
    import concourse.tile as tile

    f32 = mybir.dt.float32
    Alu = mybir.AluOpType
    Act = mybir.ActivationFunctionType

    nc = bacc.Bacc("TRN2", target_bir_lowering=False, debug=False)

    refb = nc.dram_tensor("refb", [H, W * C], f32, kind="ExternalInput")
    # tap vectors, replicated across partitions: col (s*4+t)*C + c
    taps = nc.dram_tensor("taps", [H, S * 4 * C], f32, kind="ExternalInput")
    # rot_xyz planes: [s*3 + k, h, w]
    rxyz = nc.dram_tensor("rxyz", [S * 3, H, W], f32, kind="ExternalInput")
    # trans scalars replicated across partitions: col s*3 + k
    tvec = nc.dram_tensor("tvec", [H, 8], f32, kind="ExternalInput")
    dep = nc.dram_tensor("dep", [DQ, H, W], f32, kind="ExternalInput")
    out = nc.dram_tensor("out", [DQ, G, H, W], f32, kind="ExternalOutput")

    with tile.TileContext(nc) as tc:
        with (
            tc.tile_pool(name="static", bufs=1) as ps,
            tc.tile_pool(name="scratch", bufs=2) as px_pool,
            tc.tile_pool(name="small", bufs=2) as pw,
            tc.tile_pool(name="acc", bufs=2) as pa,
        ):
            refb_t = ps.tile([H, W * C], f32, tag="refb")
            nc.sync.dma_start(refb_t[:], refb[:])
            taps_t = ps.tile([H, S * 4 * C], f32, tag="taps")
            nc.sync.dma_start(taps_t[:], taps[:])
            rxyz_t = ps.tile([H, S * 3 * W], f32, tag="rxyz")
            nc.sync.dma_start(
                rxyz_t[:].rearrange("h (i w) -> h i w", i=S * 3),
                rxyz[:].rearrange("i h w -> h i w"),
            )
            tvec_t = ps.tile([H, 8], f32, tag="tvec")
            nc.sync.dma_start(tvec_t[:], tvec[:])
            ones_t = ps.tile([H, W], f32, tag="ones")
            nc.vector.memset(ones_t[:], 1.0)

            # ---- DOT build (depth-independent, hoisted): DOT_st[p, g, w] ----
            dots = {}
            for s in range(S):
                for t in range(4):
                    p1 = px_pool.tile([H, W * C], f32, tag="p1", name="p1")
                    nc.vector.tensor_tensor(
                        p1[:].rearrange("p (w c) -> p w c", c=C),
                        refb_t[:].rearrange("p (w c) -> p w c", c=C),
                        taps_t[:, (s * 4 + t) * C : (s * 4 + t + 1) * C]
                        .unsqueeze(1)
                        .to_broadcast([H, W, C]),
                        Alu.mult,
                    )
                    dt_ = ps.tile([H, G * W], f32, tag=f"dot{s}{t}", name=f"dot{s}{t}")
                    # out layout [p, g, w] (g outer), src iterated (w, g, cc)
                    nc.vector.tensor_reduce(
                        dt_[:].rearrange("p (g w) -> p w g", g=G),
                        p1[:].rearrange("p (w g c) -> p w g c", g=G, c=CPG),
                        mybir.AxisListType.X,
                        Alu.add,
                    )
                    dots[(s, t)] = dt_

            for d in range(DQ):
                dep_t = pw.tile([H, W], f32, tag="dep")
                nc.sync.dma_start(dep_t[:], dep[d, :, :])

                simacc = pa.tile([H, G * W], f32, tag="simacc")
                first = True
                for v in range(S):
                    rx = [
                        rxyz_t[:, (v * 3 + k) * W : (v * 3 + k + 1) * W]
                        for k in range(3)
                    ]
                    tb = [tvec_t[:, v * 3 + k : v * 3 + k + 1] for k in range(3)]

                    X = pw.tile([H, W], f32, tag="X")
                    Y = pw.tile([H, W], f32, tag="Y")
                    Z = pw.tile([H, W], f32, tag="Z")
                    nc.vector.tensor_tensor(X[:], rx[0], dep_t[:], Alu.mult)
                    nc.vector.tensor_tensor(Y[:], rx[1], dep_t[:], Alu.mult)
                    nc.vector.tensor_tensor(Z[:], rx[2], dep_t[:], Alu.mult)
                    nc.scalar.activation(X[:], X[:], Act.Identity, bias=tb[0], scale=1.0)
                    nc.scalar.activation(Y[:], Y[:], Act.Identity, bias=tb[1], scale=1.0)
                    nc.scalar.activation(Z[:], Z[:], Act.Identity, bias=tb[2], scale=1.0)

                    # zpos handling: X*=m, Y*=m, Z = m ? Z : 1 (bit-exact)
                    import concourse.mybir as _mb

                    zm = pw.tile([H, W], f32, tag="zm")
                    zi = pw.tile([H, W], _mb.dt.int8, tag="zi")
                    nc.vector.tensor_scalar(zm[:], Z[:], 0.001, None, Alu.is_gt)
                    nc.vector.tensor_scalar(zi[:], Z[:], 0.001, None, Alu.is_le)
                    nc.vector.tensor_tensor(X[:], X[:], zm[:], Alu.mult)
                    nc.vector.tensor_tensor(Y[:], Y[:], zm[:], Alu.mult)
                    nc.vector.copy_predicated(Z[:], zi[:], ones_t[:])

                    rZ = pw.tile([H, W], f32, tag="rZ")
                    nc.vector.reciprocal(rZ[:], Z[:])
                    fx = pw.tile([H, W], f32, tag="fx")
                    fy = pw.tile([H, W], f32, tag="fy")
                    nc.vector.tensor_tensor(fx[:], X[:], rZ[:], Alu.mult)
                    nc.vector.tensor_tensor(fy[:], Y[:], rZ[:], Alu.mult)

                    # in-bounds masking (px<W, px>=0); px in [0,1) here so
                    # fx = px - floor(px) = px directly
                    nc.vector.scalar_tensor_tensor(
                        fx[:], fx[:], float(W), fx[:], Alu.is_lt, Alu.mult
                    )
                    nc.vector.scalar_tensor_tensor(
                        fx[:], fx[:], 0.0, fx[:], Alu.is_ge, Alu.mult
                    )
                    nc.vector.scalar_tensor_tensor(
                        fy[:], fy[:], float(H), fy[:], Alu.is_lt, Alu.mult
                    )
                    nc.vector.scalar_tensor_tensor(
                        fy[:], fy[:], 0.0, fy[:], Alu.is_ge, Alu.mult
                    )

                    gx = pw.tile([H, W], f32, tag="gx")
                    gy = pw.tile([H, W], f32, tag="gy")
                    nc.vector.tensor_scalar(gx[:], fx[:], -1.0, 1.0, Alu.mult, Alu.add)
                    nc.vector.tensor_scalar(gy[:], fy[:], -1.0, 1.0, Alu.mult, Alu.add)

                    # weights for taps [(0,0), (0,1), (1,0), (1,1)]
                    wts = [
                        pw.tile([H, W], f32, tag=f"w{t}", name=f"w{t}")
                        for t in range(4)
                    ]
                    nc.vector.tensor_tensor(wts[0][:], gx[:], gy[:], Alu.mult)
                    nc.vector.tensor_tensor(wts[1][:], fx[:], gy[:], Alu.mult)
                    nc.vector.tensor_tensor(wts[2][:], gx[:], fy[:], Alu.mult)
                    nc.vector.tensor_tensor(wts[3][:], fx[:], fy[:], Alu.mult)

                    for t in range(4):
                        wb = (
                            wts[t][:]
                            .unsqueeze(1)
                            .to_broadcast([H, G, W])
                        )
                        dsl = dots[(v, t)][:].rearrange("p (g w) -> p g w", g=G)
                        if first:
                            nc.vector.tensor_tensor(
                                simacc[:].rearrange("p (g w) -> p g w", g=G),
                                dsl,
                                wb,
                                Alu.mult,
                            )
                            first = False
                        else:
                            tmp = pw.tile([H, G * W], f32, tag="tmp")
                            nc.vector.tensor_tensor(
                                tmp[:].rearrange("p (g w) -> p g w", g=G),
                                dsl,
                                wb,
                                Alu.mult,
                            )
                            nc.vector.tensor_tensor(
                                simacc[:], simacc[:], tmp[:], Alu.add
                            )

                nc.sync.dma_start(
                    out[d, :, :, :].rearrange("g p w -> p g w"),
                    simacc[:].rearrange("p (g w) -> p g w", g=G),
                )

    nc.compile()
    _PROGRAM_CACHE["nc"] = nc
    return nc


def _host_prep(ref_feature, src_features, ref_proj, src_projs, depth_sample):
    """Projection-matrix chain bit-matched to the reference via jax CPU."""
    import jax
    import jax.numpy as jnp

    rot_xyz_all = np.zeros((S, B, 3, H, W), np.float32)
    trans_all = np.zeros((S, B, 3), np.float32)
    with jax.default_device(jax.devices("cpu")[0]):
        intr = jnp.asarray(INTR)
        intr_inv = jnp.asarray(INTR_INV)
        ref_p = intr_inv @ jnp.asarray(np.asarray(ref_proj))[:, :3, :4]  # [B,3,4]
        yy, xx = jnp.meshgrid(
            jnp.arange(H, dtype=jnp.float32), jnp.arange(W, dtype=jnp.float32),
            indexing="ij",
        )
        xyz = jnp.stack([xx.ravel(), yy.ravel(), jnp.ones(H * W, jnp.float32)])
        for s in range(S):
            src_p = intr_inv @ jnp.asarray(np.asarray(src_projs)[s])[:, :3, :4]
            proj = jnp.einsum("bij,bkj->bik", src_p[:, :, :3], ref_p[:, :, :3])
            trans = intr @ (src_p[:, :, 3:4] - proj @ ref_p[:, :, 3:4])
            rot = intr @ proj @ intr_inv
            rot_xyz = rot @ xyz  # [B,3,HW]
            rot_xyz_all[s] = np.asarray(rot_xyz).reshape(B, 3, H, W)
            trans_all[s] = np.asarray(trans).reshape(B, 3)

    # tap vectors: the 2x2 corner footprint of each (s,b) source image
    feats = np.asarray(src_features)
    tapv = np.zeros((S, B, 4, C), np.float32)
    for ti, (ty, tx) in enumerate(((0, 0), (0, 1), (1, 0), (1, 1))):
        tapv[:, :, ti, :] = feats[:, :, :, ty % H, tx % W]

    refb = (np.asarray(ref_feature).transpose(0, 2, 3, 1) * np.float32(0.25)).reshape(
        B, H, W * C
    )
    return rot_xyz_all, trans_all, tapv, refb


def _check_degenerate(rot_xyz, trans, dep):
    """Verify px,py in [0,1) for all pixels/planes/views (float32 mirror of the
    device computation, conservative margin). Returns True iff the fast path is
    valid for these inputs."""
    for s in range(S):
        for b in range(B):
            rx = rot_xyz[s, b]
            t = trans[s, b]
            dq = dep[b]
            Z = rx[2] * dq + t[2]
            zm = (Z > 0.001).astype(np.float32)
            Zc = np.where(Z > 0.001, Z, np.float32(1.0))
            for k, lim in ((0, W), (1, H)):
                P = (rx[k] * dq + t[k]) * zm / Zc
                P = P * ((P < lim) & (P >= 0))
                if P.max() >= 0.999 or P.min() < 0.0:
                    return False
    return True


def _fallback_numpy(rot_xyz, trans, tapv_unused, refb, dep, src_features):
    """General (gather-based) host computation, used only if the degenerate
    fast-path assumption fails for the given inputs."""
    feats = np.asarray(src_features)
    P = np.ascontiguousarray(feats.transpose(0, 1, 3, 4, 2))  # [S,B,H,W,C]
    Px = np.roll(P, -1, axis=3)
    Py = np.roll(P, -1, axis=2)
    Pxy = np.roll(Py, -1, axis=3)
    tabs = np.concatenate([P, Px, Py, Pxy], axis=-1).reshape(S, B, HW, 4 * C)
    full = np.zeros((B, G, D, H, W), np.float32)
    for b in range(B):
        refb_b = refb[b].reshape(H, W, C)
        simacc = np.zeros((D, H, W, G), np.float32)
        for v in range(S):
            rx = rot_xyz[v, b][:, None]
            t = trans[v, b]
            dq = dep[b]
            X = rx[0] * dq + t[0]
            Y = rx[1] * dq + t[1]
            Z = rx[2] * dq + t[2]
            zm = (Z > 0.001).astype(np.float32)
            X, Y = X * zm, Y * zm
            Zc = np.where(Z > 0.001, Z, np.float32(1.0))
            px = X / Zc
            py = Y / Zc
            px = px * ((px < W) & (px >= 0)).astype(np.float32)
            py = py * ((py < H) & (py >= 0)).astype(np.float32)
            fx = px - np.floor(px)
            fy = py - np.floor(py)
            x0 = px - fx
            y0 = py - fy
            gx = np.float32(1.0) - fx
            gy = np.float32(1.0) - fy
            wts = [gx * gy, fx * gy, gx * fy, fx * fy]
            idx = (y0 * W + x0).astype(np.int32)
            gat = tabs[v, b][idx]
            R = (
                gat.reshape(D, H, W, 4, G, CPG)
                * refb_b.reshape(1, H, W, 1, G, CPG)
            ).sum(axis=-1)
            simacc += sum(R[:, :, :, ti, :] * wts[ti][..., None] for ti in range(4))
        full[b] = simacc.transpose(3, 0, 1, 2)
    return full


def _make_in_maps(ref_feature, src_features, ref_proj, src_projs, depth_sample):
    rot_xyz, trans, tapv, refb = _host_prep(
        ref_feature, src_features, ref_proj, src_projs, depth_sample
    )
    dep = np.asarray(depth_sample)
    if not _check_degenerate(rot_xyz, trans, dep):
        return None, (rot_xyz, trans, tapv, refb, dep)

    in_maps = []
    for k in range(NCORES):
        b, q = k // 4, k % 4
        rx = rot_xyz[:, b].reshape(S * 3, H, W)
        tv = np.zeros((H, 8), np.float32)
        tv[:, 0:3] = trans[0, b]
        tv[:, 3:6] = trans[1, b]
        tp = np.broadcast_to(
            tapv[:, b].reshape(1, S * 4 * C), (H, S * 4 * C)
        ).copy()
        in_maps.append(
            {
                "refb": refb[b],
                "taps": tp,
                "rxyz": np.ascontiguousarray(rx),
                "tvec": tv,
                "dep": np.ascontiguousarray(dep[b, q * DQ : (q + 1) * DQ]),
            }
        )
    return in_maps, None


def kernel(ref_feature, src_features, ref_proj, src_projs, depth_sample):
    from concourse.bass_utils import run_bass_kernel_spmd

    in_maps, fb = _make_in_maps(
        ref_feature, src_features, ref_proj, src_projs, depth_sample
    )
    if in_maps is None:
        rot_xyz, trans, tapv, refb, dep = fb
        return _fallback_numpy(rot_xyz, trans, tapv, refb, dep, src_features)

    nc = _build_program()
    res = run_bass_kernel_spmd(nc, in_maps, core_ids=list(range(NCORES)))

    full = np.zeros((B, G, D, H, W), np.float32)
    for k in range(NCORES):
        b, q = k // 4, k % 4
        full[b, :, q * DQ : (q + 1) * DQ] = res.results[k]["out"].transpose(1, 0, 2, 3)
    return full
